# revision 6
# baseline (speedup 1.0000x reference)
"""Causal self-attention (B=4, T=2048, C=1024, H=16) on 8 Trainium2 NeuronCores.

Core index = 2*batch + head_group: each core owns one batch element and 8 of
the 16 heads (tensor-parallel split of c_attn output dim / c_proj input dim).
Each core emits a partial projection out^T [C, T] in fp16; the host sums the
two head-group partials per batch and adds the bias terms.

v3: single software-pipelined instruction stream keeping the PE continuously
busy (PE idle resets the Tensor p-state to half clock for ~3us):

  - Startup: warm-up matmuls hold the PE busy from the preamble on, while
    the serialized DMA chain loads critical tensors first (W_qk half, x^T
    tn0) so real chains start ~13us in.
  - Pre-phase: qk/v projection chains for t-chunks tn0, tn1 (PE-dense).
  - Attention stream: 160 score-groups (2 j-tiles each) emitted flat; attV
    trails scores by SKEW groups so exp (ACT) has slack. Causal masking of
    diagonal blocks is a DVE multiply of the exp output by a lower-tri tile
    (no PE negI matmuls).
  - qk/v chains for tn2/tn3 and out-proj chains metered into the stream as
    PE filler matched to the ACT exp deficit per ic-band; 3 proj chains are
    reserved to cover the pipeline drain before the final proj(tn3) tail.
  - Normalize: rowsum staged psum->SBUF, fast reciprocal, gpsimd partition
    broadcast, multiply straight from PSUM; emission delayed one group to
    keep reads clear of the accumulating matmul's write tail.

fp16 datapath, fp32 PSUM accumulation, fp32 softmax denominator; fp16 output
partials (the host reduction adds them in float64).
"""

import numpy as np

import concourse.bass as bass
import concourse.mybir as mybir
import concourse.tile as tile
from concourse import bacc, bass_utils

B, T, C, H = 4, 2048, 1024, 16
HD = C // H          # 64 head dim
N_CORES = 8
HG = H // 2          # 8 heads per core
CL = HG * HD         # 512 local width of q/k/v
TT = T // 128        # 16 t-tiles
CB = C // 128        # 8 c-tiles
DB = CL // 128       # 4 local-hd tiles
NIC = T // 512       # i-chunks (4)
SKEW = 3             # attV trails scores by this many groups
NWARM = 110          # warm-up matmuls covering the input-DMA window

f32 = mybir.dt.float32
f16 = mybir.dt.float16

_PROG_CACHE = {}


def _emit(tc, aps):
    nc = tc.nc
    Exp = mybir.ActivationFunctionType.Exp
    Copy = mybir.ActivationFunctionType.Copy

    x_ap = aps["x"]
    wqk_ap = aps["wqk"]
    wv_ap = aps["wv"]
    wp_ap = aps["wp"]
    bqk_ap = aps["bqk"]
    outT_ap = aps["outT"]

    from contextlib import ExitStack

    with ExitStack() as st:
        const = st.enter_context(tc.tile_pool(name="const", bufs=1))
        p_xT = st.enter_context(tc.tile_pool(name="xT", bufs=1))
        p_qkT = st.enter_context(tc.tile_pool(name="qkT", bufs=1))
        p_v = st.enter_context(tc.tile_pool(name="vv", bufs=1))
        p_yT = st.enter_context(tc.tile_pool(name="yT", bufs=1))
        p_w = st.enter_context(tc.tile_pool(name="wsb", bufs=1))
        p_pt = st.enter_context(tc.tile_pool(name="pt", bufs=6))
        p_rb = st.enter_context(tc.tile_pool(name="rb", bufs=4))
        p_ot = st.enter_context(tc.tile_pool(name="ot", bufs=3))
        # PSUM budget (8 banks): scores 2x[128,2,512] (4) + U' 2x[65,512] (2)
        # + qkv/proj chains 2x[128,512] (2)
        ps_sc = st.enter_context(tc.tile_pool(name="ps_sc", bufs=2, space="PSUM"))
        ps_u = st.enter_context(tc.tile_pool(name="ps_u", bufs=2, space="PSUM"))
        ps_mm = st.enter_context(tc.tile_pool(name="ps_mm", bufs=2, space="PSUM"))

        wqk_sb = p_w.tile([128, CB, CB * 128], f16)  # [c-part, cb, co*128+q]
        wv_sb = p_w.tile([128, CB, CL], f16)
        wp_sb = p_w.tile([128, DB, C], f16)
        xTn = {
            tn: p_xT.tile([128, CB, 512], f16, tag=f"xT{tn}", name=f"xT{tn}")
            for tn in range(NIC)
        }
        tril = const.tile([128, 128], f16)   # 1 where j <= i (valid)
        bqk = const.tile([128, CB], f32)
        wu = const.tile([128, 128], f16)

        # --- warm-up: keep PE busy (ramping the p-state) while DMAs land --
        nc.vector.memset(wu[:], 0.0)
        wups = ps_mm.tile([128, 512], f32, tag="mm", name="mm")
        for i in range(NWARM):
            nc.tensor.matmul(wups[:, 0:128], wu[:], wu[:], start=True, stop=True)

        # --- input DMAs: one serialized chain, critical-first -------------
        wqk_r = wqk_ap.rearrange("(cb p) n -> p cb n", p=128)
        nc.sync.dma_start(wqk_sb[:, :, 0:512], wqk_r[:, :, 0:512])
        nc.sync.dma_start_transpose(xTn[0][:, 0:4, :], x_ap[0:512, 0:512])
        nc.sync.dma_start_transpose(xTn[0][:, 4:8, :], x_ap[0:512, 512:1024])
        nc.gpsimd.dma_start(bqk[:], bqk_ap.rearrange("co p -> p co"))
        nc.gpsimd.dma_start(tril[:], aps["tril"])
        nc.sync.dma_start(wqk_sb[:, :, 512:1024], wqk_r[:, :, 512:1024])
        nc.sync.dma_start(wv_sb[:], wv_ap.rearrange("(cb p) n -> p cb n", p=128))
        for tn in (1, 2):
            nc.sync.dma_start_transpose(
                xTn[tn][:, 0:4, :], x_ap[tn * 512 : (tn + 1) * 512, 0:512]
            )
            nc.sync.dma_start_transpose(
                xTn[tn][:, 4:8, :], x_ap[tn * 512 : (tn + 1) * 512, 512:1024]
            )
        nc.sync.dma_start(wp_sb[:], wp_ap.rearrange("(db p) c -> p db c", p=128))
        nc.sync.dma_start_transpose(xTn[3][:, 0:4, :], x_ap[1536:2048, 0:512])
        nc.sync.dma_start_transpose(xTn[3][:, 4:8, :], x_ap[1536:2048, 512:1024])

        # --- persistent tiles ---------------------------------------------
        qkT = {}
        for co in range(CB):
            for tn in range(NIC):
                qkT[(co, tn)] = p_qkT.tile(
                    [128, 512], f16, tag=f"qkT_{co}_{tn}", name=f"qkT_{co}_{tn}"
                )
        vv = {}
        for jt in range(TT):
            vv[jt] = p_v.tile([128, HG, HD + 1], f16, tag=f"vv_{jt}", name=f"vv_{jt}")
            nc.vector.memset(vv[jt][:, :, HD : HD + 1], 1.0)
        yTn = {}
        for tn in range(NIC):
            yTn[tn] = p_yT.tile([128, DB, 512], f16, tag=f"yT_{tn}", name=f"yT_{tn}")

        # --- chain emitters -----------------------------------------------
        def qk_chain(tn, co):
            ps = ps_mm.tile([128, 512], f32, tag="mm", name="mm")
            for cb in range(CB):
                nc.tensor.matmul(
                    ps[:],
                    wqk_sb[:, cb, co * 128 : (co + 1) * 128],
                    xTn[tn][:, cb, :],
                    start=(cb == 0),
                    stop=(cb == CB - 1),
                )
            nc.vector.tensor_scalar_add(qkT[(co, tn)][:], ps[:], bqk[:, co : co + 1])

        def v_chain(tt):
            ps = ps_mm.tile([128, CL], f32, tag="mm", name="mm")
            for cb in range(CB):
                nc.tensor.matmul(
                    ps[:],
                    xTn[tt // 4][:, cb, (tt % 4) * 128 : (tt % 4 + 1) * 128],
                    wv_sb[:, cb, :],
                    start=(cb == 0),
                    stop=(cb == CB - 1),
                )
            nc.scalar.activation(
                vv[tt][:, :, 0:HD], ps.rearrange("p (h d) -> p h d", d=HD), Copy
            )

        def proj_chain(tn, co):
            psp = ps_mm.tile([128, 512], f32, tag="mm", name="mm")
            for db in range(DB):
                nc.tensor.matmul(
                    psp[:],
                    wp_sb[:, db, co * 128 : (co + 1) * 128],
                    yTn[tn][:, db, :],
                    start=(db == 0),
                    stop=(db == DB - 1),
                )
            ot = p_ot.tile([128, 512], f16, tag="ot", name="ot")
            nc.vector.tensor_copy(ot[:], psp[:])
            nc.sync.dma_start(
                outT_ap[co * 128 : (co + 1) * 128, tn * 512 : (tn + 1) * 512], ot[:]
            )

        # --- pre-phase: qk/v for tn0, tn1 ---------------------------------
        for tn in (0, 1):
            for co in range(CB):
                qk_chain(tn, co)
            for u in range(4):
                v_chain(4 * tn + u)

        # --- attention stream ---------------------------------------------
        groups = []  # (ic, h, [jt, jt+1])
        for ic in range(NIC):
            for h in range(HG):
                jts = list(range(4 * (ic + 1)))
                for g0 in range(0, len(jts), 2):
                    groups.append((ic, h, jts[g0 : g0 + 2]))
        NG = len(groups)  # 160; ic bands end at 16 / 48 / 96 / 160

        ups = {}
        pts = {}
        norm_queue = []

        def emit_S(k):
            ic, h, jts = groups[k]
            poff = 64 * (h % 2)
            co_q = h // 2
            co_k = 4 + h // 2
            psg = ps_sc.tile([128, 2, 512], f32, tag="sc", name="sc")
            for ix, jt in enumerate(jts):
                m = jt % 4
                diag = ic == jt // 4
                lo = 128 * m if diag else 0
                nc.tensor.matmul(
                    psg[:, ix, lo:512],
                    qkT[(co_k, jt // 4)][poff : poff + 64, m * 128 : (m + 1) * 128],
                    qkT[(co_q, ic)][poff : poff + 64, lo:512],
                    start=True,
                    stop=True,
                )
            pt = p_pt.tile([128, 2, 512], f16, tag="p", name="p")
            nc.scalar.activation(pt[:], psg[:], Exp, scale=1.0 / np.sqrt(HD))
            # causal mask on diagonal blocks: zero exp(s) where j > i
            for ix, jt in enumerate(jts):
                m = jt % 4
                if ic == jt // 4:
                    lo = 128 * m
                    nc.vector.tensor_mul(
                        pt[:, ix, lo : lo + 128], pt[:, ix, lo : lo + 128], tril[:]
                    )
            pts[k] = pt

        def emit_norm(ic, h):
            poff = 64 * (h % 2)
            up = ups.pop((ic, h))
            rs = p_rb.tile([1, 512], f32, tag="rs", name="rs")
            nc.vector.tensor_copy(rs[:], up[HD : HD + 1, :])
            rr = p_rb.tile([1, 512], f32, tag="rr", name="rr")
            nc.vector.reciprocal_approx_fast(rr[:], rs[:])
            rb = p_rb.tile([HD, 512], f32, tag="rb", name="rb")
            nc.gpsimd.partition_broadcast(rb[:], rr[0:1, :], channels=HD)
            nc.vector.tensor_mul(
                yTn[ic][poff : poff + HD, h // 2, :], up[0:HD, :], rb[:]
            )

        def emit_A(k):
            # delayed normalize from the previous unit first
            while norm_queue:
                emit_norm(*norm_queue.pop(0))
            ic, h, jts = groups[k]
            pt = pts.pop(k)
            if jts[0] == 0:
                ups[(ic, h)] = ps_u.tile([HD + 1, 512], f32, tag="u", name="u")
            up = ups[(ic, h)]
            for ix, jt in enumerate(jts):
                m = jt % 4
                diag = ic == jt // 4
                lo = 128 * m if diag else 0
                nc.tensor.matmul(
                    up[:, lo:512],
                    vv[jt][:, h, :],
                    pt[:, ix, lo:512],
                    start=(jt == 0),
                    stop=(jt == 4 * ic + 3),
                )
            if jts[-1] == 4 * ic + 3:
                norm_queue.append((ic, h))

        # filler chains with deadlines: tn2 before group 48, tn3 before 96,
        # out-proj tn0-2 spread over the ic3 band (3 reserved for the drain).
        fill_sched = {k: [] for k in range(NG)}

        def _sched(k, fn):
            fill_sched[k].append(fn)

        # front-loaded: PE-per-group must exceed ACT exp-per-group everywhere
        for i, co in enumerate(range(CB)):          # tn2 qk over ic0 band
            _sched(2 * i, lambda co=co: qk_chain(2, co))
        for i, tt in enumerate((8, 9)):             # first tn2 v chains in ic0
            _sched(5 + 6 * i, lambda tt=tt: v_chain(tt))
        for i, tt in enumerate((10, 11)):
            _sched(17 + 4 * i, lambda tt=tt: v_chain(tt))
        for i, co in enumerate(range(CB)):          # tn3 qk over ic1 band
            _sched(25 + 3 * i, lambda co=co: qk_chain(3, co))
        for i, tt in enumerate((12, 13, 14, 15)):   # tn3 v early in ic2 band
            _sched(50 + 5 * i, lambda tt=tt: v_chain(tt))
        for i, co in enumerate(range(CB)):          # proj tn0 late in ic2 band
            _sched(70 + 3 * i, lambda co=co: proj_chain(0, co))
        for i, co in enumerate(range(CB)):          # proj tn1 over ic3 band
            _sched(96 + 5 * i, lambda co=co: proj_chain(1, co))
        for i, co in enumerate(range(3)):           # proj tn2 first 3; 5 drain
            _sched(136 + 5 * i, lambda co=co: proj_chain(2, co))
        drain_fill = [
            (lambda co=co: proj_chain(2, co)) for co in range(3, CB)
        ]

        for k in range(NG):
            emit_S(k)
            if k >= SKEW:
                emit_A(k - SKEW)
            for fn in fill_sched[k]:
                fn()
        for j, k in enumerate(range(NG - SKEW, NG)):
            emit_A(k)
            if j < len(drain_fill):
                drain_fill[j]()
        for fn in drain_fill[len(range(NG - SKEW, NG)) :]:
            fn()
        while norm_queue:
            emit_norm(*norm_queue.pop(0))
        for co in range(CB):
            proj_chain(3, co)


def _build_program():
    nc = bacc.Bacc("TRN2", target_bir_lowering=False, debug=False, num_devices=N_CORES)
    aps = {
        "x": nc.dram_tensor("x", [T, C], f16, kind="ExternalInput").ap(),
        "wqk": nc.dram_tensor("wqk", [C, CB * 128], f16, kind="ExternalInput").ap(),
        "wv": nc.dram_tensor("wv", [C, CL], f16, kind="ExternalInput").ap(),
        "wp": nc.dram_tensor("wp", [CL, C], f16, kind="ExternalInput").ap(),
        "bqk": nc.dram_tensor("bqk", [CB, 128], f32, kind="ExternalInput").ap(),
        "tril": nc.dram_tensor("tril", [128, 128], f16, kind="ExternalInput").ap(),
        "outT": nc.dram_tensor("outT", [C, T], f16, kind="ExternalOutput").ap(),
    }
    with tile.TileContext(nc) as tc:
        _emit(tc, aps)
    nc.compile()
    return nc


def get_program():
    if "nc" not in _PROG_CACHE:
        _PROG_CACHE["nc"] = _build_program()
    return _PROG_CACHE["nc"]


def _host_consts():
    j = np.arange(128)[:, None]
    i = np.arange(128)[None, :]
    tril = (j <= i).astype(np.float16)  # 1 = keep (j <= i)
    return tril


def make_in_maps(x, W_attn, b_attn, W_proj):
    """Build the 8 per-core input maps. Core index = 2*batch + head_group."""
    tril = _host_consts()
    in_maps = []
    for core in range(N_CORES):
        b = core // 2
        g = core % 2
        wq = W_attn[:, g * CL : (g + 1) * CL]
        wk = W_attn[:, C + g * CL : C + (g + 1) * CL]
        wqk = np.concatenate([wq, wk], axis=1)  # [C, 1024], cols = co*128+q
        wv = W_attn[:, 2 * C + g * CL : 2 * C + (g + 1) * CL]
        bqk = np.concatenate(
            [b_attn[g * CL : (g + 1) * CL], b_attn[C + g * CL : C + (g + 1) * CL]]
        ).reshape(CB, 128)
        in_maps.append(
            {
                "x": np.ascontiguousarray(x[b]).astype(np.float16),
                "wqk": np.ascontiguousarray(wqk).astype(np.float16),
                "wv": np.ascontiguousarray(wv).astype(np.float16),
                "wp": np.ascontiguousarray(W_proj[g * CL : (g + 1) * CL, :]).astype(
                    np.float16
                ),
                "bqk": np.ascontiguousarray(bqk).astype(np.float32),
                "tril": tril,
            }
        )
    return in_maps


def run(x, W_attn, b_attn, W_proj, b_proj, trace=False):
    nc = get_program()
    in_maps = make_in_maps(x, W_attn, b_attn, W_proj)
    res = bass_utils.run_bass_kernel_spmd(
        nc, in_maps, core_ids=list(range(N_CORES)), trace=trace
    )
    # combine: out[b] = sum_g outT_{2b+g}^T + (bv_g @ Wp_g summed) + b_proj
    corr = b_proj.astype(np.float64).copy()
    for g in range(2):
        bv_g = b_attn[2 * C + g * CL : 2 * C + (g + 1) * CL]
        corr += bv_g.astype(np.float64) @ W_proj[g * CL : (g + 1) * CL, :].astype(
            np.float64
        )
    out = np.empty((B, T, C), np.float32)
    for b in range(B):
        acc = (
            res.results[2 * b]["outT"].T.astype(np.float64)
            + res.results[2 * b + 1]["outT"].T.astype(np.float64)
            + corr
        )
        out[b] = acc.astype(np.float32)
    return out, res


def kernel(x, W_attn, b_attn, W_proj, b_proj):
    x = np.asarray(x, np.float32)
    W_attn = np.asarray(W_attn, np.float32)
    b_attn = np.asarray(b_attn, np.float32)
    W_proj = np.asarray(W_proj, np.float32)
    b_proj = np.asarray(b_proj, np.float32)
    out, _ = run(x, W_attn, b_attn, W_proj, b_proj)
    return out
